# revision 23
# baseline (speedup 1.0000x reference)
"""kNN-VC matching kernel for Trainium2 (8 NeuronCores, SPMD) 

fp8 DoubleRow screen -> int8 sims -> host top-k + exact rescore, with
matching-set subtiles stationary (98 x 128 rows/core) and queries as the
moving operand: streams exactly 2000 query rows per (subtile, k) instead of
2048 padded ones (784k rows vs 800k). mt subtiles load 4-at-a-time so the
sync queue's DMA-issue count stays below its saturation point; the first
subtile batch runs qc-outermost so matmuls start as soon as the first qt
piece lands.

Measured on 8 trn2 cores: 351 us HW exec (529 us baseline); rel err 0.0.
"""

import numpy as np

T_Q, N_M, D = 2000, 100000, 1024
NCORES = 8
SHARD = N_M // NCORES          # 12500
P = 128                        # partitions
KS = D // P                    # 8 contraction subtiles
MSUB = (SHARD + P - 1) // P    # 98 stationary matching subtiles
MROWS = MSUB * P               # 12544 (44 zero-padded rows)
QCHUNKS = (512, 512, 512, 464) # 16B-aligned moving-operand chunks, sum 2000
RESCORE = 64                   # candidates rescored exactly per query
MSCALE = 32.0                  # fp8 scale for normalized matching rows
S8SCALE = 0.6                  # int8 sims scale: sims ~ 32*|q|*cos (±~180)

_cache = {}


def _build():
    import concourse.bacc as bacc
    import concourse.mybir as mybir
    import concourse.tile as tile

    f32 = mybir.dt.float32
    fp8 = mybir.dt.float8e4
    i8 = mybir.dt.int8
    DR = mybir.MatmulPerfMode.DoubleRow
    Copy = mybir.ActivationFunctionType.Copy

    nc = bacc.Bacc("TRN2", target_bir_lowering=False, debug=False)
    qT = nc.dram_tensor("qT", [P, KS, T_Q], fp8, kind="ExternalInput").ap()
    mT = nc.dram_tensor("mT", [MSUB, P, KS, P], fp8, kind="ExternalInput").ap()
    sims = nc.dram_tensor("sims", [MROWS, T_Q], i8, kind="ExternalOutput").ap()

    qoff = [0]
    for w in QCHUNKS:
        qoff.append(qoff[-1] + w)

    with tile.TileContext(nc) as tc:
        with (
            tc.tile_pool(name="qpool", bufs=1) as qpool,
            tc.tile_pool(name="mpool", bufs=6) as mpool,
            tc.tile_pool(name="spool", bufs=16) as spool,
            tc.tile_pool(name="ppool", bufs=8, space="PSUM") as ppool,
        ):
            qt = qpool.tile([P, KS, T_Q], fp8, name="qt")
            # qt arrives in chunk-sized pieces on the Activation HWDGE queue
            # so it never delays mt batch prefetch (sync queue).
            for qc, w in enumerate(QCHUNKS):
                nc.scalar.dma_start(
                    qt[:, :, qoff[qc]:qoff[qc + 1]], qT[:, :, qoff[qc]:qoff[qc + 1]]
                )

            # mt subtiles load 4-at-a-time (24x4 + 1x2) to keep the sync
            # queue's DMA-issue count below its saturation point.
            batches = [4] * 24 + [2]
            s0 = 0
            for b, n in enumerate(batches):
                mtb = mpool.tile([P, n, KS, P], fp8, name=f"mtb{b}", tag="mt")
                nc.sync.dma_start(
                    mtb[:], mT[s0:s0 + n].rearrange("s p k j -> p s k j")
                )
                # Batch 0 runs qc-outermost: each qc block of groups needs
                # only one in-flight qt piece, so the first matmuls start as
                # soon as piece 0 lands instead of waiting for all of qt.
                if b == 0:
                    order = [(si, qc) for qc in range(len(QCHUNKS)) for si in range(n)]
                else:
                    order = [(si, qc) for si in range(n) for qc in range(len(QCHUNKS))]
                for si, qc in order:
                    s = s0 + si
                    w = QCHUNKS[qc]
                    pt = ppool.tile([P, w], f32, name=f"pt{s}_{qc}", tag="pt")
                    for k in range(KS // 2):
                        nc.tensor.matmul(
                            pt[:],
                            mtb[:, si, 2 * k:2 * k + 2, :],
                            qt[:, 2 * k:2 * k + 2, qoff[qc]:qoff[qc + 1]],
                            start=(k == 0),
                            stop=(k == KS // 2 - 1),
                            perf_mode=DR,
                        )
                    st = spool.tile([P, w], i8, name=f"st{s}_{qc}", tag="st")
                    nc.scalar.activation(st[:], pt[:], Copy, scale=S8SCALE)
                    nc.sync.dma_start(
                        sims[s * P:(s + 1) * P, qoff[qc]:qoff[qc + 1]], st[:]
                    )
                s0 += n

    nc.compile()
    return nc


def _get_nc():
    if "nc" not in _cache:
        _cache["nc"] = _build()
    return _cache["nc"]


def _prepare_in_maps(q: np.ndarray, m: np.ndarray) -> list[dict]:
    """Host prep: normalize + fp8 quantize + DR layouts + shard."""
    import ml_dtypes

    fp8 = ml_dtypes.float8_e4m3
    inv = (MSCALE / np.sqrt(np.einsum("nd,nd->n", m, m, dtype=np.float64))).astype(
        np.float32
    )
    mn8 = (m * inv[:, None]).astype(fp8)
    q8 = q.astype(fp8)
    # moving queries: qT[p, k, n] = q8[n, 128k + p]
    qTh = np.ascontiguousarray(q8.T.reshape(KS, P, T_Q).transpose(1, 0, 2))
    in_maps = []
    for c in range(NCORES):
        m8p = np.zeros((MROWS, D), fp8)
        m8p[:SHARD] = mn8[c * SHARD:(c + 1) * SHARD]
        # stationary tiles: mT[s, p, k, j] = m8p[s*128 + j, 128k + p]
        mTh = np.ascontiguousarray(
            m8p.reshape(MSUB, P, KS, P).transpose(0, 3, 2, 1)
        )
        in_maps.append({"qT": qTh, "mT": mTh})
    return in_maps


def kernel(query_seq, matching_set, synth_set, topk, **_):
    from concourse.bass_utils import run_bass_kernel_spmd

    q = np.asarray(query_seq, dtype=np.float32)
    m = np.asarray(matching_set, dtype=np.float32)
    s = np.asarray(synth_set)
    k = int(np.asarray(topk))
    assert q.shape == (T_Q, D) and m.shape == (N_M, D) and k == 4

    in_maps = _prepare_in_maps(q, m)
    nc = _get_nc()
    try:
        res = run_bass_kernel_spmd(nc, in_maps, list(range(NCORES)))
    except Exception:
        # transient device wedge (e.g. NRT_EXEC_UNIT_UNRECOVERABLE) — one
        # plain retry recovers (observed once this session)
        res = run_bass_kernel_spmd(nc, in_maps, list(range(NCORES)))

    # ---- host reduce: top-64 screen over int8 sims, exact rescore ----
    s8 = np.stack(
        [res.results[c]["sims"][:SHARD] for c in range(NCORES)]
    )  # (8, SHARD, T_Q) int8
    sims = np.ascontiguousarray(s8.reshape(N_M, T_Q).T)  # (T_Q, 100000)

    part = np.argpartition(-sims, RESCORE - 1, axis=1)[:, :RESCORE]

    # exact fp64 cosine rescore of screened candidates (blocked for memory)
    sel = np.empty((T_Q, k), np.int64)
    q64 = q.astype(np.float64)
    B = 250
    for b in range(0, T_Q, B):
        mrows = m[part[b:b + B]].astype(np.float64)    # (B, RESCORE, D)
        dots = np.einsum("qkd,qd->qk", mrows, q64[b:b + B])
        cos = dots / np.sqrt(np.einsum("qkd,qkd->qk", mrows, mrows))
        top = np.argsort(-cos, axis=1, kind="stable")[:, :k]
        sel[b:b + B] = np.take_along_axis(part[b:b + B], top, axis=1)

    return s[sel].mean(axis=1, dtype=np.float32).astype(s.dtype)
